# revision 6
# baseline (speedup 1.0000x reference)
"""AtomPredictor forward pass on 8 TRN2 NeuronCores.

out = relu(((atom_h[i0] + atom_h[i1]) * 0.5) @ W_h + b_h) @ W_o + b_o

Strategy:
  - atom table cast to bf16 (halves gather traffic), 0.5 folded into W_h
  - atoms partitioned into 16 buckets of 31250 rows so gpsimd.dma_gather's
    int16 indices can address them; pairs classed by (bucket(a0), bucket(a1))
    with 2-choice balancing, dealt round-robin across the 8 cores so every
    (core, class) bin holds <= 512 slots (padded with index 0)
  - per class two transposed dma_gather calls (rotating SWDGE queues 0-3)
    deliver [128=d_local, 2=dblk, 512=slot] tiles -> no on-chip transposes
  - DVE pair-add, TensorE mm1 in hT orientation (hid on partitions), fused
    relu+bias on ACT/DVE, N=1 matmuls drop output scalars into PSUM columns,
    bulk-copied + b_o once per 512 subtile-columns; host inverts the slot
    permutation at the end
"""
import numpy as np
import ml_dtypes

from concourse import bacc, bass, mybir
from concourse.bass_utils import run_bass_kernel_spmd
from concourse.tile import TileContext

F32 = mybir.dt.float32
BF16 = mybir.dt.bfloat16
I16 = mybir.dt.int16

N_ATOMS = 500000
N_PAIRS = 1000000
D = 256
HID = 256
N_CORES = 8

NB = 16                     # buckets
BUCKET = N_ATOMS // NB      # 31250
NCLS = NB * NB              # 256 ordered classes
CAP = 512                   # slots per (core, class)
GCAP = CAP * N_CORES        # 4096 global per class
SLOTS = NCLS * CAP          # 131072 per core
TS = SLOTS // 128           # 1024 subtile-columns
CIDX = CAP // 16            # idx cols per call (32)
EPOCH = 512

_nc_cache = None


def _build():
    nc = bacc.Bacc(num_swdge_queues=4)
    tbl = nc.declare_dram_parameter("tbl", [N_ATOMS, D], BF16, isOutput=False)
    idx0 = nc.declare_dram_parameter("idx0", [128, NCLS * CIDX], I16, isOutput=False)
    idx1 = nc.declare_dram_parameter("idx1", [128, NCLS * CIDX], I16, isOutput=False)
    wh = nc.declare_dram_parameter("wh", [D, HID], BF16, isOutput=False)
    bh = nc.declare_dram_parameter("bh", [128, 2], F32, isOutput=False)
    wo = nc.declare_dram_parameter("wo", [128, 2], BF16, isOutput=False)
    bo = nc.declare_dram_parameter("bo", [128, 1], F32, isOutput=False)
    out = nc.declare_dram_parameter("out", [128, TS], F32, isOutput=True)

    with TileContext(nc) as tc:
        with tc.tile_pool(name="const", bufs=1) as cpool, \
             tc.tile_pool(name="work", bufs=1) as wpool, \
             tc.tile_pool(name="psum", bufs=1, space="PSUM") as ppool:
            idx0_sb = cpool.tile([128, NCLS * CIDX], I16)
            idx1_sb = cpool.tile([128, NCLS * CIDX], I16)
            nc.sync.dma_start(out=idx0_sb[:], in_=idx0[:])
            nc.sync.dma_start(out=idx1_sb[:], in_=idx1[:])
            wh_sb0 = cpool.tile([128, HID], BF16)
            wh_sb1 = cpool.tile([128, HID], BF16)
            nc.sync.dma_start(out=wh_sb0[:], in_=wh[0:128, :])
            nc.sync.dma_start(out=wh_sb1[:], in_=wh[128:256, :])
            bh_sb = cpool.tile([128, 2], F32)
            nc.sync.dma_start(out=bh_sb[:], in_=bh[:])
            wo_sb = cpool.tile([128, 2], BF16)
            nc.sync.dma_start(out=wo_sb[:], in_=wo[:])
            bo_sb = cpool.tile([128, 1], F32)
            nc.sync.dma_start(out=bo_sb[:], in_=bo[:])
            outbuf = cpool.tile([128, TS], F32)

            o_ps_tiles = {}

            def o_ps_for(ep):
                if ep not in o_ps_tiles:
                    o_ps_tiles[ep] = ppool.tile(
                        [128, EPOCH], F32, name=f"ops_{ep}", tag="ops", bufs=2)
                return o_ps_tiles[ep]

            for c in range(NCLS):
                wa, wb = (c // NB) * BUCKET, (c % NB) * BUCKET
                g0 = wpool.tile([128, 2, CAP], BF16, name=f"g0_{c}",
                                tag="g0", bufs=3)
                g1 = wpool.tile([128, 2, CAP], BF16, name=f"g1_{c}",
                                tag="g1", bufs=3)
                nc.gpsimd.dma_gather(
                    out_ap=g0[:, :, :], in_ap=tbl[wa:wa + BUCKET, :],
                    idxs_ap=idx0_sb[:, c * CIDX:(c + 1) * CIDX],
                    num_idxs=CAP, num_idxs_reg=CAP, elem_size=D,
                    transpose=True, queue_num=(2 * c) % 4)
                nc.gpsimd.dma_gather(
                    out_ap=g1[:, :, :], in_ap=tbl[wb:wb + BUCKET, :],
                    idxs_ap=idx1_sb[:, c * CIDX:(c + 1) * CIDX],
                    num_idxs=CAP, num_idxs_reg=CAP, elem_size=D,
                    transpose=True, queue_num=(2 * c + 1) % 4)

                padd = wpool.tile([128, 2, CAP], BF16, name=f"padd_{c}",
                                  tag="padd", bufs=2)
                nc.vector.tensor_tensor(
                    out=padd[:, :, :], in0=g0[:, :, :], in1=g1[:, :, :],
                    op=mybir.AluOpType.add)

                ht_sbs = []
                for hblk in range(2):
                    ht_ps = ppool.tile([128, CAP], F32,
                                       name=f"htp_{c}_{hblk}", tag="htp", bufs=4)
                    nc.tensor.matmul(
                        out=ht_ps[:],
                        lhsT=wh_sb0[:, hblk * 128:(hblk + 1) * 128],
                        rhs=padd[:, 0, :], start=True, stop=False)
                    nc.tensor.matmul(
                        out=ht_ps[:],
                        lhsT=wh_sb1[:, hblk * 128:(hblk + 1) * 128],
                        rhs=padd[:, 1, :], start=False, stop=True)
                    ht_sb = wpool.tile([128, CAP], BF16,
                                       name=f"hts_{c}_{hblk}", tag="hts", bufs=4)
                    if hblk == 0:
                        nc.scalar.activation(
                            out=ht_sb[:], in_=ht_ps[:],
                            func=mybir.ActivationFunctionType.Relu,
                            bias=bh_sb[:, 0:1], scale=1.0)
                    else:
                        nc.vector.tensor_scalar(
                            out=ht_sb[:], in0=ht_ps[:],
                            scalar1=bh_sb[:, 1:2], scalar2=0.0,
                            op0=mybir.AluOpType.add, op1=mybir.AluOpType.max)
                    ht_sbs.append(ht_sb)

                for j in range(CAP // 128):
                    t = c * (CAP // 128) + j
                    ep, col = divmod(t, EPOCH)
                    o_ps = o_ps_for(ep)
                    for hblk in range(2):
                        nc.tensor.matmul(
                            out=o_ps[:, col:col + 1],
                            lhsT=ht_sbs[hblk][:, j * 128:(j + 1) * 128],
                            rhs=wo_sb[:, hblk:hblk + 1],
                            start=(hblk == 0), stop=(hblk == 1))
                    if col == EPOCH - 1:
                        nc.vector.tensor_scalar(
                            out=outbuf[:, ep * EPOCH:(ep + 1) * EPOCH],
                            in0=o_ps[:], scalar1=bo_sb[:, 0:1], scalar2=None,
                            op0=mybir.AluOpType.add)
            nc.sync.dma_start(out=out[:], in_=outbuf[:])

    nc.finalize()
    return nc


def _get_nc():
    global _nc_cache
    if _nc_cache is None:
        _nc_cache = _build()
    return _nc_cache


def _emulate(tbl_bf, wh_bf, b_h, wo_bf, b_o, i0, i1):
    """Host fallback matching device numerics closely (for spilled pairs)."""
    a = tbl_bf[i0].astype(np.float32) + tbl_bf[i1].astype(np.float32)
    a = a.astype(ml_dtypes.bfloat16).astype(np.float32)
    h = a @ wh_bf.astype(np.float32) + b_h
    h = np.maximum(h, 0.0).astype(ml_dtypes.bfloat16).astype(np.float32)
    return h @ wo_bf.astype(np.float32)[:, 0] + b_o[0]


def kernel(atom_h, pair_idx, W_h, b_h, W_o, b_o, _trace=False):
    atom_h = np.asarray(atom_h)
    pair_idx = np.asarray(pair_idx).astype(np.int64)
    W_h = np.asarray(W_h, dtype=np.float32)
    b_h = np.asarray(b_h, dtype=np.float32)
    W_o = np.asarray(W_o, dtype=np.float32)
    b_o = np.asarray(b_o, dtype=np.float32)

    tbl = atom_h.astype(ml_dtypes.bfloat16)
    wh = (0.5 * W_h).astype(ml_dtypes.bfloat16)
    bh = b_h.reshape(2, 128).T.copy()
    wo = W_o[:, 0].reshape(2, 128).T.astype(ml_dtypes.bfloat16).copy()
    bo = np.full((128, 1), b_o[0], dtype=np.float32)

    # ---- class assignment: 2-choice halving, global ----
    a0, a1 = pair_idx[:, 0], pair_idx[:, 1]
    b0, b1 = a0 // BUCKET, a1 // BUCKET
    lo, hi = np.minimum(b0, b1), np.maximum(b0, b1)
    ukey = lo * NB + hi
    order = np.argsort(ukey, kind="stable")

    cls = np.empty(N_PAIRS, dtype=np.int64)      # ordered class per pair
    side0 = np.empty(N_PAIRS, dtype=np.int64)    # atom gathered on side0
    side1 = np.empty(N_PAIRS, dtype=np.int64)
    spill_mask = np.zeros(N_PAIRS, dtype=bool)

    uk_sorted = ukey[order]
    bounds = np.searchsorted(uk_sorted, np.arange(NB * NB + 1))
    for k in range(NB * NB):
        s, e = bounds[k], bounds[k + 1]
        if s == e:
            continue
        idxs = order[s:e]
        i, j = k // NB, k % NB
        n = e - s
        if i == j:
            take = min(n, GCAP)
            cls[idxs[:take]] = i * NB + j
            side0[idxs[:take]] = a0[idxs[:take]]
            side1[idxs[:take]] = a1[idxs[:take]]
            if take < n:
                spill_mask[idxs[take:]] = True
        else:
            half = min((n + 1) // 2, GCAP)
            other = min(n - half, GCAP)
            first, second = idxs[:half], idxs[half:half + other]
            if half + other < n:
                spill_mask[idxs[half + other:]] = True
            # class (i, j): side0 from bucket i
            cls[first] = i * NB + j
            swap = b0[first] != i
            side0[first] = np.where(swap, a1[first], a0[first])
            side1[first] = np.where(swap, a0[first], a1[first])
            # class (j, i): side0 from bucket j
            cls[second] = j * NB + i
            swap = b0[second] != j
            side0[second] = np.where(swap, a1[second], a0[second])
            side1[second] = np.where(swap, a0[second], a1[second])

    valid = ~spill_mask
    vp = np.nonzero(valid)[0]
    # deal pairs of each class across cores / slots
    o2 = np.argsort(cls[vp], kind="stable")
    vp = vp[o2]
    ccls = cls[vp]
    cb = np.searchsorted(ccls, np.arange(NCLS + 1))
    core_of = np.empty(len(vp), dtype=np.int64)
    slot_of = np.empty(len(vp), dtype=np.int64)
    for cidx in range(NCLS):
        s, e = cb[cidx], cb[cidx + 1]
        n = e - s
        r = np.arange(n)
        core_of[s:e] = r % N_CORES
        slot_of[s:e] = cidx * CAP + r // N_CORES

    # ---- build per-core index planes & slot->pair map ----
    in_maps = []
    slot_pair = np.full((N_CORES, SLOTS), -1, dtype=np.int64)
    loc0 = (side0[vp] % BUCKET).astype(np.int16)
    loc1 = (side1[vp] % BUCKET).astype(np.int16)
    for core in range(N_CORES):
        m = core_of == core
        sl = slot_of[m]
        i0_plane = np.zeros(SLOTS, dtype=np.int16)
        i1_plane = np.zeros(SLOTS, dtype=np.int16)
        i0_plane[sl] = loc0[m]
        i1_plane[sl] = loc1[m]
        slot_pair[core][sl] = vp[m]
        # slot i of class c -> idx_arr[16*q + i%16, c*CIDX + i//16] for q=0..7
        blk0 = i0_plane.reshape(NCLS, CIDX, 16)   # [c, i//16, i%16]
        blk1 = i1_plane.reshape(NCLS, CIDX, 16)
        arr0 = np.tile(blk0.transpose(2, 0, 1).reshape(16, NCLS * CIDX), (8, 1))
        arr1 = np.tile(blk1.transpose(2, 0, 1).reshape(16, NCLS * CIDX), (8, 1))
        in_maps.append({
            "tbl": tbl, "idx0": np.ascontiguousarray(arr0),
            "idx1": np.ascontiguousarray(arr1),
            "wh": wh, "bh": bh, "wo": wo, "bo": bo,
        })

    nc = _get_nc()
    res = run_bass_kernel_spmd(nc, in_maps, core_ids=list(range(N_CORES)),
                               trace=_trace)

    full = np.empty(N_PAIRS, dtype=np.float32)
    for core in range(N_CORES):
        o = np.asarray(res.results[core]["out"], dtype=np.float32)
        # out[p, t]: slot i of class c -> subtile t = c*4 + i//128, p = i%128
        dev = o.reshape(128, TS)
        flat = dev.T.reshape(NCLS, CAP // 128, 128).reshape(SLOTS)
        sp = slot_pair[core]
        m = sp >= 0
        full[sp[m]] = flat[m]

    if spill_mask.any():
        sp_idx = np.nonzero(spill_mask)[0]
        full[sp_idx] = _emulate(tbl, wh, b_h, wo, b_o,
                                pair_idx[sp_idx, 0], pair_idx[sp_idx, 1])

    out_full = full[:, None]
    if _trace:
        return out_full, res
    return out_full
